# revision 19
# baseline (speedup 1.0000x reference)
"""MoE (top-2 routed + 2 shared experts, SwiGLU) Trainium2 kernel, 8 NeuronCores.

Sharding / schedule (v3):
  - Routed experts: expert-parallel, 2 experts per core (E=16 over 8 cores),
    capacity 2304 (mean 2048 + 6sigma; graceful drop on overflow).
  - Shared experts: DATA-parallel (each core runs both shared experts over its
    own 2048 tokens, full H). Shared expert s0 runs BEFORE the routed phase
    (covers gate/AllGather/compaction latency); s1 runs AFTER, overlapping the
    ReduceScatter of the routed buffer. The 0.5 mean factor is folded into w2.
  - Gate: data-parallel over token shards, AllGathered (tiny).
  - Combine: routed scatter-adds go to a zero-initialized (N, D) bf16 buffer;
    ReduceScatter sums across cores (overlapped with s1); final out =
    s0_local + s1_local + rs_shard.

Queue discipline (the collective occupies the sync engine for its whole
duration, so anything that must run concurrently with the ReduceScatter is
issued from the scalar or gpsimd queues):
  - sync:   input/weight loads up to the routed phase, rs_out loads, final out
  - scalar: gate softmax Exp, silu, all s1-phase loads/stores
  - gpsimd: collectives, rbuf zeroing, compaction scatters, gathers/scatter-adds

Numerics: FFN matmuls in bf16 with fp32 PSUM accumulation; gate in fp32
(routing decisions are selection-sensitive).
"""

import numpy as np

B, T, D, H, E, K, S = 4, 4096, 1024, 2048, 16, 2, 2
N = B * T              # 16384 tokens
NCORES = 8
EPC = E // NCORES      # 2 routed experts per core
NSH = N // NCORES      # 2048 tokens per shard
CAP = 2304             # per-expert capacity (mean 2048 + 6 sigma)
TBLK = 512             # token block
NB_SH = NSH // TBLK    # 4 shared blocks (local tokens)
BIG = 1.0e9            # OOB sentinel for scatter positions
MULTI_SCATTER = False  # one indirect DMA per expert (vs 128 per expert)

_CACHE = {}


def _build():
    import concourse.bacc as bacc
    import concourse.bass as bass
    import concourse.mybir as mybir
    import concourse.tile as tile
    from concourse.masks import make_upper_triangular

    dt = mybir.dt
    AF = mybir.ActivationFunctionType
    ALU = mybir.AluOpType

    nc = bacc.Bacc("TRN2", target_bir_lowering=False, debug=False,
                   num_devices=NCORES)

    # ---- I/O ----
    xg_d = nc.dram_tensor("xg", [D, NSH], dt.float32, kind="ExternalInput")
    xtl_d = nc.dram_tensor("xtl", [D, NSH], dt.bfloat16, kind="ExternalInput")
    xr_d = nc.dram_tensor("xr", [N, D], dt.bfloat16, kind="ExternalInput")
    gw_d = nc.dram_tensor("gw", [D, E], dt.float32, kind="ExternalInput")
    gb_d = nc.dram_tensor("gb", [128, E], dt.float32, kind="ExternalInput")
    es_d = nc.dram_tensor("esel", [EPC, 128, 16, 2 * E], dt.float32, kind="ExternalInput")
    s13_d = nc.dram_tensor("sw13", [S, 8, 128, 2 * H], dt.bfloat16, kind="ExternalInput")
    s2_d = nc.dram_tensor("sw2", [S, 16, 128, D], dt.bfloat16, kind="ExternalInput")
    e13_d = nc.dram_tensor("ew13", [EPC, 8, 128, 2 * H], dt.bfloat16, kind="ExternalInput")
    e2_d = nc.dram_tensor("ew2", [EPC, 16, 128, D], dt.bfloat16, kind="ExternalInput")
    out_d = nc.dram_tensor("out", [NSH, D], dt.bfloat16, kind="ExternalOutput")

    RG = [list(range(NCORES))]

    from contextlib import ExitStack
    with tile.TileContext(nc) as tc:
        with ExitStack() as ctx:
            dram = ctx.enter_context(tc.tile_pool(name="dram", bufs=1, space="DRAM"))
            cns = ctx.enter_context(tc.tile_pool(name="const", bufs=1))
            sg = ctx.enter_context(tc.tile_pool(name="gate", bufs=2))
            sxg_g = ctx.enter_context(tc.tile_pool(name="xgt", bufs=1))
            sxl = ctx.enter_context(tc.tile_pool(name="xtl", bufs=2))
            se = ctx.enter_context(tc.tile_pool(name="ext", bufs=2))
            scm = ctx.enter_context(tc.tile_pool(name="cmp", bufs=1))
            smt = ctx.enter_context(tc.tile_pool(name="mts", bufs=1))
            sy = ctx.enter_context(tc.tile_pool(name="ys", bufs=1))
            syh = ctx.enter_context(tc.tile_pool(name="ysh", bufs=2))
            ssi = ctx.enter_context(tc.tile_pool(name="silu", bufs=2))
            swe = ctx.enter_context(tc.tile_pool(name="wexp", bufs=1))
            sxr = ctx.enter_context(tc.tile_pool(name="gxr", bufs=2))
            sxr1 = ctx.enter_context(tc.tile_pool(name="gxr1", bufs=1))
            sst = ctx.enter_context(tc.tile_pool(name="strm", bufs=2))
            srs = ctx.enter_context(tc.tile_pool(name="rstr", bufs=1))
            psc = ctx.enter_context(tc.tile_pool(name="psc", bufs=2, space="PSUM"))
            psh = ctx.enter_context(tc.tile_pool(name="psh", bufs=4, space="PSUM"))
            psy = ctx.enter_context(tc.tile_pool(name="psy", bufs=2, space="PSUM"))

            # ---------- DRAM temporaries ----------
            ag_in = dram.tile([NSH, 2 * E], dt.float32)
            ag_out = dram.tile([N, 2 * E], dt.float32, addr_space="Shared")
            pairs = [dram.tile([CAP, 2], dt.float32, name=f"pairs{i}")
                     for i in range(EPC)]
            rbuf = dram.tile([N, D], dt.bfloat16)
            rs_out = dram.tile([NSH, D], dt.bfloat16)
            ybuf = dram.tile([NSH, D], dt.bfloat16)

            rbv = rbuf.rearrange("(c p) d -> p c d", p=128)
            ybv = ybuf.rearrange("(c p) d -> p c d", p=128)
            rsv = rs_out.rearrange("(c p) d -> p c d", p=128)
            ov = out_d.rearrange("(c p) d -> p c d", p=128)
            xgv = xg_d.rearrange("(c p) n -> p c n", p=128)
            xlv = xtl_d.rearrange("(c p) n -> p c n", p=128)
            agv = ag_out.rearrange("(t p) e -> p t e", p=128)

            # ---------- constants ----------
            gw_sb = cns.tile([128, 8, E], dt.float32)
            nc.sync.dma_start(gw_sb[:], gw_d.rearrange("(c p) e -> p c e", p=128))
            gb_sb = cns.tile([128, E], dt.float32)
            nc.sync.dma_start(gb_sb[:], gb_d[:])
            es_sb = cns.tile([128, EPC, 16, 2 * E], dt.float32)
            nc.sync.dma_start(es_sb[:], es_d.rearrange("l p t e -> p l t e"))
            su = cns.tile([128, 128], dt.float32)
            make_upper_triangular(nc, su[:], val=1.0, diag=False)  # 1 iff row < col
            ones_col = cns.tile([128, 1], dt.float32)
            nc.vector.memset(ones_col[:], 1.0)
            tok_i = scm.tile([128, 128], dt.int32, tag="of", name="tok_i")
            nc.gpsimd.iota(tok_i[:], pattern=[[128, 128]], base=0,
                           channel_multiplier=1)
            tok_f = cns.tile([128, 128], dt.float32)
            nc.vector.tensor_copy(tok_f[:], tok_i[:])
            wslab = cns.tile([128, EPC, 128], dt.float32)
            mslab = cns.tile([128, EPC, 128], dt.float32)
            idx16 = cns.tile([128, EPC, CAP // 16], dt.int16)
            wsc = cns.tile([128, EPC, CAP // 128], dt.float32)
            zb = cns.tile([128, 2, D], dt.bfloat16)
            nc.vector.memset(zb[:], 0.0)

            # per-chunk expert weight tiles (single-buffered; per-chunk WAR
            # lets the next expert's DMA start as each chunk's last read
            # retires). Used for: s0, e0, e1, s1 in sequence.
            def load_expert_w(which, le, dma=nc.sync):
                w13_d, w2_d = (s13_d, s2_d) if which == "s" else (e13_d, e2_d)
                e13c = []
                for dc in range(8):
                    t13 = swe.tile([128, 2 * H], dt.bfloat16, tag=f"e13_{dc}",
                                   name=f"e13c{which}{le}_{dc}")
                    dma.dma_start(t13[:], w13_d[le, dc])
                    e13c.append(t13)
                e2c = []
                for hb in range(16):
                    t2 = swe.tile([128, D], dt.bfloat16, tag=f"e2_{hb}",
                                  name=f"e2c{which}{le}_{hb}")
                    dma.dma_start(t2[:], w2_d[le, hb])
                    e2c.append(t2)
                return e13c, e2c

            # ---------- P1: gate on local token shard (fp32) ----------
            for tb in range(NSH // 128):
                xgt = sxg_g.tile([128, 8, 128], dt.float32, tag="xg")
                nc.sync.dma_start(xgt[:], xgv[:, :, tb * 128:(tb + 1) * 128])
                pg = psc.tile([128, E], dt.float32, tag="pc")
                for dc in range(8):
                    nc.tensor.matmul(pg[:], lhsT=xgt[:, dc, :], rhs=gw_sb[:, dc, :],
                                     start=(dc == 0), stop=(dc == 7))
                logits = sg.tile([128, E], dt.float32, tag="lg")
                nc.vector.tensor_copy(logits[:], pg[:])
                mx8 = sg.tile([128, 8], dt.float32, tag="mx")
                nc.vector.max(mx8[:], logits[:])
                negmx = sg.tile([128, 1], dt.float32, tag="nm")
                nc.vector.tensor_scalar(negmx[:], mx8[:, 0:1], -1.0, None,
                                        op0=ALU.mult)
                exps = sg.tile([128, E], dt.float32, tag="ex")
                nc.scalar.activation(exps[:], logits[:], AF.Exp,
                                     bias=negmx[:, 0:1], scale=1.0)
                ssum = sg.tile([128, 1], dt.float32, tag="ss")
                nc.vector.tensor_reduce(ssum[:], exps[:], axis=mybir.AxisListType.X,
                                        op=ALU.add)
                rcp = sg.tile([128, 1], dt.float32, tag="rc")
                nc.vector.reciprocal(rcp[:], ssum[:])
                scores = sg.tile([128, E], dt.float32, tag="sc")
                nc.vector.tensor_scalar(scores[:], exps[:], rcp[:, 0:1], None,
                                        op0=ALU.mult)
                nc.vector.tensor_add(scores[:], scores[:], gb_sb[:])
                smax = sg.tile([128, 8], dt.float32, tag="sm")
                nc.vector.max(smax[:], scores[:])
                mask = sg.tile([128, E], dt.float32, tag="mk")
                nc.vector.tensor_tensor(
                    out=mask[:], in0=scores[:],
                    in1=smax[:, 1:2].to_broadcast([128, E]), op=ALU.is_ge)
                wmat = sg.tile([128, E], dt.float32, tag="wm")
                nc.vector.tensor_mul(wmat[:], logits[:], mask[:])
                # stores issued from the scalar queue: keeps the sync queue
                # free for the s0 weight stream
                nc.scalar.dma_start(ag_in[tb * 128:(tb + 1) * 128, 0:E], wmat[:])
                nc.scalar.dma_start(ag_in[tb * 128:(tb + 1) * 128, E:2 * E], mask[:])

            # s0 weights after the gate loads on the sync queue
            sw_p = load_expert_w("s", 0)

            # a few rbuf-zero DMAs, then the AllGather trigger, then the rest
            # (all on gpsimd so the sync queue stays clear)
            for i in range(8):
                nc.gpsimd.dma_start(rbv[:, 2 * i:2 * (i + 1), :], zb[:])
            nc.gpsimd.collective_compute(
                "AllGather", ALU.bypass, replica_groups=RG,
                ins=[ag_in[:]], outs=[ag_out[:]])
            for i in range(8, 64):
                nc.gpsimd.dma_start(rbv[:, 2 * i:2 * (i + 1), :], zb[:])

            # ================= FFN block builder (uniform) ==================
            def ffn_block(e13c, e2c, rhs_ap, blen, out_fn):
                mtr = smt.tile([128, 16, blen], dt.bfloat16, tag="mt",
                               padded_shape=[128, 16, TBLK])
                for hb in range(16):
                    ph1 = psh.tile([128, blen], dt.float32, tag="ph",
                                   padded_shape=[128, TBLK])
                    ph3 = psh.tile([128, blen], dt.float32, tag="ph",
                                   padded_shape=[128, TBLK])
                    for dc in range(8):
                        nc.tensor.matmul(
                            ph1[:], lhsT=e13c[dc][:, hb * 128:(hb + 1) * 128],
                            rhs=rhs_ap[:, dc, :], start=(dc == 0), stop=(dc == 7))
                    for dc in range(8):
                        nc.tensor.matmul(
                            ph3[:], lhsT=e13c[dc][:, H + hb * 128:H + (hb + 1) * 128],
                            rhs=rhs_ap[:, dc, :], start=(dc == 0), stop=(dc == 7))
                    sil = ssi.tile([128, blen], dt.float32, tag="si",
                                   padded_shape=[128, TBLK])
                    nc.scalar.activation(sil[:], ph1[:], AF.Silu)
                    nc.vector.tensor_mul(mtr[:, hb, :], sil[:], ph3[:])
                for t4 in range(blen // 128):
                    for dh in range(2):
                        py = psy.tile([128, 512], dt.float32)
                        for hb in range(16):
                            nc.tensor.matmul(
                                py[:], lhsT=mtr[:, hb, t4 * 128:(t4 + 1) * 128],
                                rhs=e2c[hb][:, dh * 512:(dh + 1) * 512],
                                start=(hb == 0), stop=(hb == 15))
                        out_fn(py, t4, dh)

            ysh_ctr = [0]

            # shared block: xtb preloaded by caller; per-t4 out chunks
            def shared_block(xtb, e13c, e2c, make_fn, dest_fn):
                state = {}

                def out_fn(py, t4, dh):
                    if dh == 0:
                        ysh_ctr[0] += 1
                        state[t4] = syh.tile(
                            [128, 1, D], dt.bfloat16, tag="ysh",
                            name=f"ysh{ysh_ctr[0]}")
                    make_fn(state[t4], py, t4, dh)
                    if dh == 1:
                        dest_fn(state[t4], t4)
                ffn_block(e13c, e2c, xtb[:], TBLK, out_fn)

            xtb_ctr = [0]

            def load_xtb(blk, dma=nc.sync):
                xtb_ctr[0] += 1
                xtb = sxl.tile([128, 8, TBLK], dt.bfloat16, tag="xtl",
                               name=f"xtb{xtb_ctr[0]}")
                dma.dma_start(xtb[:], xlv[:, :, blk * TBLK:(blk + 1) * TBLK])
                return xtb

            s13c, s2c = sw_p

            def s0_make(yo, py, t4, dh):
                nc.vector.tensor_copy(yo[:, 0, dh * 512:(dh + 1) * 512], py[:])

            def s0_block(blk, xtb):
                shared_block(
                    xtb, s13c, s2c, s0_make,
                    lambda yo, t4, blk=blk: nc.sync.dma_start(
                        ybv[:, 4 * blk + t4:4 * blk + t4 + 1, :], yo[:]))

            # ---------- s0 block 0 ----------
            xtb0 = load_xtb(0)
            s0_block(0, xtb0)

            # ---------- P3: slab extraction ----------
            for ts in range(8):
                agc = se.tile([128, 16, 2 * E], dt.float32, tag="ag",
                              name=f"agc{ts}")
                nc.sync.dma_start(agc[:], agv[:, ts * 16:(ts + 1) * 16, :])
                for le in range(EPC):
                    for hm, slab in ((0, wslab), (1, mslab)):
                        tmp = se.tile([128, 16, E], dt.float32, tag="p3t",
                                      name=f"p3t{ts}_{le}_{hm}")
                        nc.gpsimd.tensor_mul(
                            tmp[:],
                            agc[:, :, hm * E:(hm + 1) * E],
                            es_sb[:, le, :, hm * E:(hm + 1) * E])
                        nc.vector.tensor_reduce(
                            slab[:, le, ts * 16:(ts + 1) * 16], tmp[:],
                            axis=mybir.AxisListType.X, op=ALU.add)

            # ---------- s0 block 1 (+ preload xtb for blocks 2,3) ----------
            xtb1 = load_xtb(1)
            s0_block(1, xtb1)
            xtb2 = load_xtb(2)
            xtb3 = load_xtb(3)

            # ---------- P4: compaction per expert ----------
            def compact(le):
                pcs = psc.tile([128, 1], dt.float32, tag="pc", name=f"pcs{le}")
                nc.tensor.matmul(pcs[:], lhsT=mslab[:, le, :], rhs=ones_col[:],
                                 start=True, stop=True)
                csum = scm.tile([128, 1], dt.float32, tag="cs", name=f"cs{le}")
                nc.vector.tensor_copy(csum[:], pcs[:])
                pos = psc.tile([128, 128], dt.float32, tag="pc", name=f"pos{le}")
                # pos[p,t] = sum_{c<t} csum[c] + sum_{p'<p} mask[p',t]
                nc.tensor.matmul(pos[:], lhsT=csum[:, 0:1].to_broadcast([128, 128]),
                                 rhs=su[:], start=True, stop=False)
                nc.tensor.matmul(pos[:], lhsT=su[:], rhs=mslab[:, le, :],
                                 start=False, stop=True)
                bigm = scm.tile([128, 128], dt.float32, tag="bg", name=f"bg{le}")
                nc.gpsimd.tensor_scalar(bigm[:], mslab[:, le, :], -BIG, BIG,
                                        op0=ALU.mult, op1=ALU.add)
                posv = scm.tile([128, 128], dt.float32, tag="pv", name=f"pv{le}")
                nc.vector.tensor_mul(posv[:], pos[:], mslab[:, le, :])
                posf = scm.tile([128, 128], dt.float32, tag="pf", name=f"pf{le}")
                nc.gpsimd.tensor_add(posf[:], posv[:], bigm[:])
                offs = scm.tile([128, 128], dt.int32, tag="of", name=f"of{le}")
                nc.gpsimd.tensor_copy(offs[:], posf[:])
                wtok = scm.tile([128, 128, 2], dt.float32, tag="wt", name=f"wt{le}")
                nc.gpsimd.tensor_copy(wtok[:, :, 0], tok_f[:])
                nc.gpsimd.tensor_copy(wtok[:, :, 1], wslab[:, le, :])
                zp = scm.tile([128, CAP // 128, 2], dt.float32, tag="zp",
                              name=f"zp{le}")
                nc.gpsimd.memset(zp[:], 0.0)
                nc.sync.dma_start(
                    pairs[le].rearrange("(c p) e -> p c e", p=128), zp[:])
                if MULTI_SCATTER:
                    nc.gpsimd.indirect_dma_start(
                        out=pairs[le][:],
                        out_offset=bass.IndirectOffsetOnAxis(
                            ap=offs[:, :], axis=0),
                        in_=wtok[:, :, :], in_offset=None,
                        bounds_check=CAP - 1, oob_is_err=False)
                else:
                    for t in range(128):
                        nc.gpsimd.indirect_dma_start(
                            out=pairs[le][:],
                            out_offset=bass.IndirectOffsetOnAxis(
                                ap=offs[:, t:t + 1], axis=0),
                            in_=wtok[:, t, :], in_offset=None,
                            bounds_check=CAP - 1, oob_is_err=False)

                # wrapped int16 index table (16-wrap, replicated to 8 stripes)
                idxf = scm.tile([128, CAP // 16], dt.float32, tag="ix",
                                name=f"ix{le}")
                for k in range(8):
                    nc.sync.dma_start(
                        idxf[16 * k:16 * (k + 1), :],
                        pairs[le].rearrange("(c s) e -> s c e", s=16)[:, :, 0])
                nc.gpsimd.tensor_copy(idx16[:, le, :], idxf[:])
                nc.sync.dma_start(
                    wsc[:, le, :],
                    pairs[le].rearrange("(c p) e -> p c e", p=128)[:, :, 1])

            RBLK = [512, 512, 512, 512, 256]
            ROFF = [0, 512, 1024, 1536, 2048]

            def routed_gather(le, blk):
                blen = RBLK[blk]
                off = ROFF[blk]
                pool = sxr if blen == TBLK else sxr1
                xgT = pool.tile([128, 8, blen], dt.bfloat16, tag=f"xgT{blen}",
                                name=f"xgT{le}_{blk}")
                nc.gpsimd.dma_gather(
                    out_ap=xgT[:], in_ap=xr_d[:],
                    idxs_ap=idx16[:, le, off // 16:(off + blen) // 16],
                    num_idxs=blen, num_idxs_reg=blen,
                    elem_size=D, transpose=True)
                return xgT

            compact(0)
            # prefetch first two gathers of expert 0 right after its tables
            g_pre = [routed_gather(0, 0), routed_gather(0, 1)]

            # ---------- s0 block 2 ----------
            s0_block(2, xtb2)

            compact(1)
            ew_p = load_expert_w("e", 0)

            # ---------- s0 block 3 ----------
            s0_block(3, xtb3)

            # ---------- P6: routed experts ----------
            for le in range(EPC):
                e13c, e2c = ew_p if le == 0 else load_expert_w("e", le)
                for blk in range(len(RBLK)):
                    blen = RBLK[blk]
                    off = ROFF[blk]
                    if le == 0 and blk < 2:
                        xgT = g_pre[blk]
                    else:
                        xgT = routed_gather(le, blk)
                    ysb = sy.tile([128, blen // 128, D], dt.bfloat16, tag="ysb",
                                  padded_shape=[128, 4, D],
                                  name=f"ysb{le}_{blk}")

                    def out_fn(py, t4, dh, le=le, off=off, ysb=ysb):
                        wcol = wsc[:, le, off // 128 + t4:off // 128 + t4 + 1]
                        nc.vector.tensor_scalar(
                            ysb[:, t4, dh * 512:(dh + 1) * 512], py[:],
                            wcol, None, op0=ALU.mult)
                    ffn_block(e13c, e2c, xgT[:], blen, out_fn)
                    nc.gpsimd.dma_scatter_add(
                        out_ap=rbuf[:], in_ap=ysb[:],
                        idxs_ap=idx16[:, le, off // 16:(off + blen) // 16],
                        num_idxs=blen, num_idxs_reg=blen, elem_size=D)

            # ---------- P7: ReduceScatter (overlaps s1) ----------
            nc.gpsimd.collective_compute(
                "ReduceScatter", ALU.add, replica_groups=RG,
                ins=[rbuf[:]], outs=[rs_out[:]])

            # ---------- s1: second shared expert + combine ----------
            # all s1 loads/stores go through the scalar queue: the sync queue
            # is blocked for the duration of the ReduceScatter.
            s13c, s2c = load_expert_w("s", 1, dma=nc.scalar)
            xtb_s1 = [load_xtb(blk, dma=nc.scalar) for blk in range(2)]
            for blk in range(NB_SH):
                xtb = xtb_s1[blk] if blk < 2 else load_xtb(blk, dma=nc.scalar)
                ybt = [None, None]
                for hf in range(2):
                    yb = sst.tile([128, 2, D], dt.bfloat16, tag="yb",
                                  name=f"yb{blk}_{hf}")
                    nc.scalar.dma_start(
                        yb[:], ybv[:, 4 * blk + 2 * hf:4 * blk + 2 * hf + 2, :])
                    ybt[hf] = yb
                if blk < 2:
                    # park s0+s1 back into ybuf; rs added later
                    def mk(yo, py, t4, dh, ybt=ybt):
                        nc.vector.tensor_tensor(
                            out=yo[:, 0, dh * 512:(dh + 1) * 512], in0=py[:],
                            in1=ybt[t4 // 2][:, t4 % 2, dh * 512:(dh + 1) * 512],
                            op=ALU.add)

                    def dst(yo, t4, blk=blk):
                        nc.scalar.dma_start(
                            ybv[:, 4 * blk + t4:4 * blk + t4 + 1, :], yo[:])
                else:
                    # rs is ready by the time these blocks finish: fold it in
                    rst = [None, None]
                    for hf in range(2):
                        rs = srs.tile([128, 2, D], dt.bfloat16, tag="rs",
                                      name=f"rs{blk}_{hf}")
                        nc.sync.dma_start(
                            rs[:],
                            rsv[:, 4 * blk + 2 * hf:4 * blk + 2 * hf + 2, :])
                        rst[hf] = rs

                    def mk(yo, py, t4, dh, ybt=ybt, rst=rst):
                        nc.vector.tensor_tensor(
                            out=yo[:, 0, dh * 512:(dh + 1) * 512], in0=py[:],
                            in1=ybt[t4 // 2][:, t4 % 2, dh * 512:(dh + 1) * 512],
                            op=ALU.add)
                        nc.vector.tensor_tensor(
                            out=yo[:, 0, dh * 512:(dh + 1) * 512],
                            in0=yo[:, 0, dh * 512:(dh + 1) * 512],
                            in1=rst[t4 // 2][:, t4 % 2, dh * 512:(dh + 1) * 512],
                            op=ALU.add)

                    def dst(yo, t4, blk=blk):
                        nc.sync.dma_start(
                            ov[:, 4 * blk + t4:4 * blk + t4 + 1, :], yo[:])
                shared_block(xtb, s13c, s2c, mk, dst)

            # blocks 0,1 final combine: (s0+s1 from ybuf) + rs -> out
            for blk in range(2):
                for hf in range(2):
                    yb = sst.tile([128, 2, D], dt.bfloat16, tag="yb",
                                  name=f"fyb{blk}_{hf}")
                    nc.sync.dma_start(
                        yb[:], ybv[:, 4 * blk + 2 * hf:4 * blk + 2 * hf + 2, :])
                    rs = srs.tile([128, 2, D], dt.bfloat16, tag="rs",
                                  name=f"frs{blk}_{hf}")
                    nc.sync.dma_start(
                        rs[:], rsv[:, 4 * blk + 2 * hf:4 * blk + 2 * hf + 2, :])
                    nc.vector.tensor_tensor(out=yb[:], in0=yb[:], in1=rs[:],
                                            op=ALU.add)
                    nc.sync.dma_start(
                        ov[:, 4 * blk + 2 * hf:4 * blk + 2 * hf + 2, :], yb[:])

    nc.compile()
    return nc


def _prep_inputs(inputs):
    import ml_dtypes
    bf16 = ml_dtypes.bfloat16

    x = np.ascontiguousarray(np.asarray(inputs["x"], np.float32).reshape(N, D))
    gw = np.asarray(inputs["gate_w"], np.float32)
    gb = np.asarray(inputs["gate_b"], np.float32)
    ew1 = np.asarray(inputs["ew1"], np.float32)
    ew3 = np.asarray(inputs["ew3"], np.float32)
    ew2 = np.asarray(inputs["ew2"], np.float32)
    sw1 = np.asarray(inputs["sw1"], np.float32)
    sw3 = np.asarray(inputs["sw3"], np.float32)
    sw2 = np.asarray(inputs["sw2"], np.float32)

    xr = x.astype(bf16)                                       # (N, D)
    gb_b = np.broadcast_to(gb, (128, E)).copy()

    # shared experts, full weights (same for every core)
    s13 = np.empty((S, 8, 128, 2 * H), np.float32)
    s2w = np.empty((S, 16, 128, D), np.float32)
    for s in range(S):
        cat = np.concatenate([sw1[s], sw3[s]], axis=1)        # (D, 2H)
        s13[s] = cat.reshape(8, 128, 2 * H)
        s2w[s] = (sw2[s] * 0.5).reshape(16, 128, D)
    s13 = s13.astype(bf16)
    s2w = s2w.astype(bf16)

    in_maps = []
    for c in range(NCORES):
        e13 = np.empty((EPC, 8, 128, 2 * H), np.float32)
        e2c = np.empty((EPC, 16, 128, D), np.float32)
        esel = np.zeros((EPC, 128, 16, 2 * E), np.float32)
        for le in range(EPC):
            ei = c * EPC + le
            cat = np.concatenate([ew1[ei], ew3[ei]], axis=1)  # (D, 2H)
            e13[le] = cat.reshape(8, 128, 2 * H)
            e2c[le] = ew2[ei].reshape(16, 128, D)
            esel[le, :, :, ei] = 1.0
            esel[le, :, :, E + ei] = 1.0
        xl = x[c * NSH:(c + 1) * NSH]
        xg = np.ascontiguousarray(xl.T)                       # (D, NSH) fp32
        xtl = xg.astype(bf16)                                 # (D, NSH) bf16
        in_maps.append({
            "xg": xg, "xtl": xtl, "xr": xr, "gw": gw, "gb": gb_b,
            "esel": esel, "sw13": s13, "sw2": s2w,
            "ew13": e13.astype(bf16), "ew2": e2c.astype(bf16),
        })
    return in_maps


def kernel(**inputs):
    from concourse.bass_utils import run_bass_kernel_spmd

    if "nc" not in _CACHE:
        _CACHE["nc"] = _build()
    nc = _CACHE["nc"]
    in_maps = _prep_inputs(inputs)
    res = run_bass_kernel_spmd(nc, in_maps, core_ids=list(range(NCORES)))
    _CACHE["last_result"] = res
    out = np.concatenate([res.results[c]["out"] for c in range(NCORES)], axis=0)
    return out.astype(np.float32).reshape(B, T, D)
